# revision 15
# baseline (speedup 1.0000x reference)
"""Trainium2 Bass kernel for nn_BranchDiversity.

Computes, for x: [n=8, B=128, C=512, H=7, W=7]:
  xm   = mean(x, axis=C)                       -> [n, B, H, W]
  sq   = sum((xm_i - xm_j)^2, axis=(H,W))      -> [n, n, B]
  snm  = mean(exp(-GAMMA*sq), axis=B) * (1-I)  -> [n, n]
  out  = (sum(snm), -det(snm), -logdet(snm) if det>0 else nan)

Sharding: data-parallel over B across 8 NeuronCores (16 batch each).
Per core the kernel does the memory-bound channel-mean reduction
([128 partitions = (b_local, n) b-major, 25088 free = (C, H*W)]) plus the
tiny pairwise exp tail, emitting exp(-GAMMA*sq)[(b_local, i), j] = [128, 8].
Host averages partials over all 128 batches and does the 8x8 det/logdet.
"""

import numpy as np
from contextlib import ExitStack

import concourse.bass as bass
import concourse.tile as tile
from concourse import bacc, mybir
from concourse.bass_utils import run_bass_kernel_spmd

# Problem constants (hardcoded per contract; kernel.py must be self-contained)
N_CORES = 8
N = 8          # branches
B = 128        # batch
C = 512        # channels
HW = 49        # H*W = 7*7
B_SH = B // N_CORES        # 16 batch per core
P = N * B_SH               # 128 partitions = (b_local, n), b-major
CF = C * HW                # 25088 free elems per partition
GAMMA = 10.0
# exp(-GAMMA * sq_ref) with sq_ref = sq_rawsum / C^2 (xm kept as raw channel
# sums on device; 1/C^2 folded into the activation scale, exact: C^2 = 2^18)
EXP_SCALE = -GAMMA / (C * C)

# Channel chunking: big chunks for DMA efficiency, tapering trailing chunks
# so the final (exposed, serial) reduces on the critical path are short.
# Taper chosen by searching the cost-model chain (see dev notes).
CHUNKS = [48] * 7 + [24] * 3 + [16] + [12] * 4 + [10] + [8] * 3 + [6]
assert sum(CHUNKS) == C

F32 = mybir.dt.float32


def build_nc():
    """Build the per-core Bass program (SPMD: same program, different data)."""
    nc = bacc.Bacc("TRN2", target_bir_lowering=False)
    x = nc.dram_tensor("x", [P, CF], F32, kind="ExternalInput")
    e_out = nc.dram_tensor("e_out", [P, N], F32, kind="ExternalOutput")

    with tile.TileContext(nc) as tc, ExitStack() as ctx:
        _body(ctx, tc, x.ap(), e_out.ap())
    nc.compile()
    return nc


def _body(ctx, tc, x, e_out):
    nc = tc.nc

    xpool = ctx.enter_context(tc.tile_pool(name="xin", bufs=4))
    ppool = ctx.enter_context(tc.tile_pool(name="parts", bufs=2))
    spool = ctx.enter_context(tc.tile_pool(name="small", bufs=1))
    dpool = ctx.enter_context(tc.tile_pool(name="dram", bufs=1, space="DRAM"))

    # ---- Stage 1: channel-sum reduction (memory bound) ----
    # Running accumulation: reduce chunk -> part, add into acc (adds hidden
    # under DMA; only the last reduce+add are on the critical path).
    xm_sum = spool.tile([P, HW], F32)
    coff = 0
    for k, cc in enumerate(CHUNKS):
        xt = xpool.tile([P, cc * HW], F32, tag=f"xt{cc}")
        nc.sync.dma_start(out=xt, in_=x[:, coff * HW:(coff + cc) * HW])
        # view [p, hw, c] (c strided by HW) and reduce innermost (c)
        xt_v = xt.rearrange("p (c h) -> p h c", c=cc)
        if k == 0:
            nc.vector.reduce_sum(out=xm_sum, in_=xt_v,
                                 axis=mybir.AxisListType.X)
        else:
            part = ppool.tile([P, HW], F32, tag="part")
            nc.vector.reduce_sum(out=part, in_=xt_v,
                                 axis=mybir.AxisListType.X)
            nc.vector.tensor_add(xm_sum, xm_sum, part)
        coff += cc

    # ---- Stage 2: pairwise tail (tiny) ----
    # Partition layout is (b, n) b-major, so dumping xm_sum row-major to DRAM
    # directly yields the [b, j, hw] layout the gather needs (plain 2-dim copy).
    xm_dram = dpool.tile([P, HW], F32)
    nc.sync.dma_start(out=xm_dram, in_=xm_sum)

    # Bt[(b,i), (j,hw)] = xm[b, j, hw]  (broadcast over i via step-0 dim)
    Bt = spool.tile([P, N, HW], F32)
    gather_src = bass.AP(
        tensor=xm_dram.tensor,
        offset=xm_dram.offset,
        ap=[[N * HW, B_SH], [0, N], [1, N * HW]],  # (b, i bcast, (j,hw))
    )
    nc.sync.dma_start(out=Bt, in_=gather_src)

    # A[(b,i), (j,hw)] = xm[(b,i), hw] broadcast over j (stride-0 free dim)
    a_ap = xm_sum[:]
    A = bass.AP(tensor=a_ap.tensor, offset=a_ap.offset,
                ap=[list(a_ap.ap[0]), [0, N], [1, HW]])

    d = spool.tile([P, N, HW], F32)
    nc.vector.tensor_sub(d, A, Bt)
    dsq = spool.tile([P, N, HW], F32)
    nc.vector.tensor_mul(dsq, d, d)

    sq_t = spool.tile([P, N], F32)
    nc.vector.reduce_sum(out=sq_t, in_=dsq, axis=mybir.AxisListType.X)

    e_t = spool.tile([P, N], F32)
    nc.scalar.activation(e_t, sq_t, mybir.ActivationFunctionType.Exp,
                         scale=EXP_SCALE)

    nc.sync.dma_start(out=e_out, in_=e_t)


_NC_CACHE = {}


def _get_nc():
    if "nc" not in _NC_CACHE:
        _NC_CACHE["nc"] = build_nc()
    return _NC_CACHE["nc"]


def _shard(x, c):
    """Core c's input: [b_local*8 + n, C*HW] (b-major partition order)."""
    xs = x[:, c * B_SH:(c + 1) * B_SH]              # [N, B_SH, C, H, W]
    return np.ascontiguousarray(
        xs.transpose(1, 0, 2, 3, 4).reshape(P, CF))


def kernel(x: np.ndarray):
    """Full-input entry point: x [8, 128, 512, 7, 7] f32 -> (direct, det, logdet)."""
    x = np.asarray(x, dtype=np.float32)
    assert x.shape == (N, B, C, 7, 7), x.shape
    nc = _get_nc()

    in_maps = [{"x": _shard(x, c)} for c in range(N_CORES)]
    res = run_bass_kernel_spmd(nc, in_maps, core_ids=list(range(N_CORES)))

    # e_out rows are (b_local, i), cols j -> snm[i, j] = mean over all batches
    acc = np.zeros((N, N), dtype=np.float64)
    for c in range(N_CORES):
        e = res.results[c]["e_out"].astype(np.float64)  # [128, 8]
        acc += e.reshape(B_SH, N, N).sum(axis=0)        # [i, j]
    snm = acc / B
    snm *= 1.0 - np.eye(N)

    direct_div = snm.sum()
    det = np.linalg.det(snm)
    det_div = -det
    sign, logabs = np.linalg.slogdet(snm)
    logdet_div = -logabs if sign > 0 else np.float64(np.nan)

    return (
        np.float32(direct_div),
        np.float32(det_div),
        np.float32(logdet_div),
    )


# revision 19
# speedup vs baseline: 1.0496x; 1.0496x over previous
"""Trainium2 Bass kernel for nn_BranchDiversity.

Computes, for x: [n=8, B=128, C=512, H=7, W=7]:
  xm   = mean(x, axis=C)                       -> [n, B, H, W]
  sq   = sum((xm_i - xm_j)^2, axis=(H,W))      -> [n, n, B]
  snm  = mean(exp(-GAMMA*sq), axis=B) * (1-I)  -> [n, n]
  out  = (sum(snm), -det(snm), -logdet(snm) if det>0 else nan)

Sharding: data-parallel over B across 8 NeuronCores (16 batch each).
Per core the kernel does the memory-bound channel-mean reduction
([128 partitions = (b_local, n) b-major, 25088 free = (C, H*W)]) plus the
tiny pairwise exp tail, emitting exp(-GAMMA*sq)[(b_local, i), j] = [128, 8].
Host averages partials over all 128 batches and does the 8x8 det/logdet.
"""

import numpy as np
from contextlib import ExitStack

import concourse.bass as bass
import concourse.tile as tile
from concourse import bacc, mybir
from concourse.bass_utils import run_bass_kernel_spmd

# Problem constants (hardcoded per contract; kernel.py must be self-contained)
N_CORES = 8
N = 8          # branches
B = 128        # batch
C = 512        # channels
HW = 49        # H*W = 7*7
B_SH = B // N_CORES        # 16 batch per core
P = N * B_SH               # 128 partitions = (b_local, n), b-major
CF = C * HW                # 25088 free elems per partition
GAMMA = 10.0
# exp(-GAMMA * sq_ref) with sq_ref = sq_rawsum / C^2 (xm kept as raw channel
# sums on device; 1/C^2 folded into the activation scale, exact: C^2 = 2^18)
EXP_SCALE = -GAMMA / (C * C)

# Channel chunking: big chunks for DMA efficiency, tapering trailing chunks
# so the final (exposed, serial) reduces on the critical path are short.
# Taper chosen by searching the cost-model chain (see dev notes).
CHUNKS = [48] * 7 + [24] * 3 + [16] + [12] * 4 + [10] + [8] * 3 + [6]
assert sum(CHUNKS) == C

F32 = mybir.dt.float32


def build_nc():
    """Build the per-core Bass program (SPMD: same program, different data)."""
    nc = bacc.Bacc("TRN2", target_bir_lowering=False)
    x = nc.dram_tensor("x", [P, CF], F32, kind="ExternalInput")
    e_out = nc.dram_tensor("e_out", [P, N - 1], F32, kind="ExternalOutput")

    with tile.TileContext(nc) as tc, ExitStack() as ctx:
        _body(ctx, tc, x.ap(), e_out.ap())
    nc.compile()
    return nc


def _body(ctx, tc, x, e_out):
    nc = tc.nc

    xpool = ctx.enter_context(tc.tile_pool(name="xin", bufs=4))
    ppool = ctx.enter_context(tc.tile_pool(name="parts", bufs=2))
    spool = ctx.enter_context(tc.tile_pool(name="small", bufs=1))

    # ---- Stage 1: channel-sum reduction (memory bound) ----
    # Running accumulation: reduce chunk -> part, add into acc (adds hidden
    # under DMA; only the last reduce+add are on the critical path).
    xm_sum = spool.tile([P, HW], F32)
    coff = 0
    for k, cc in enumerate(CHUNKS):
        xt = xpool.tile([P, cc * HW], F32, tag=f"xt{cc}")
        nc.sync.dma_start(out=xt, in_=x[:, coff * HW:(coff + cc) * HW])
        # view [p, hw, c] (c strided by HW) and reduce innermost (c)
        xt_v = xt.rearrange("p (c h) -> p h c", c=cc)
        if k == 0:
            nc.vector.reduce_sum(out=xm_sum, in_=xt_v,
                                 axis=mybir.AxisListType.X)
        else:
            part = ppool.tile([P, HW], F32, tag="part")
            nc.vector.reduce_sum(out=part, in_=xt_v,
                                 axis=mybir.AxisListType.X)
            nc.vector.tensor_add(xm_sum, xm_sum, part)
        coff += cc

    # ---- Stage 2: pairwise tail (tiny, all on-chip) ----
    # Partition layout is (b, n) b-major: each batch's 8 branch rows sit
    # within one 32-partition stream_shuffle quadrant, so rotating branches
    # by r gives xs[(b,i), :] = xm[(b,(i+r)%8), :] without any DMA.
    R = N - 1  # rotations 1..7 cover all off-diagonal pairs
    xs = spool.tile([P, R, HW], F32)
    for r in range(1, N):
        mask = [(i & 24) | ((i + r) & 7) for i in range(32)]
        nc.vector.stream_shuffle(xs[:, r - 1, :], xm_sum, mask)

    # A[(b,i), (r,hw)] = xm[(b,i), hw] broadcast over r (stride-0 free dim)
    a_ap = xm_sum[:]
    A = bass.AP(tensor=a_ap.tensor, offset=a_ap.offset,
                ap=[list(a_ap.ap[0]), [0, R], [1, HW]])

    d = spool.tile([P, R, HW], F32)
    nc.vector.tensor_sub(d, A, xs)
    dsq = spool.tile([P, R, HW], F32)
    nc.vector.tensor_mul(dsq, d, d)

    sq_t = spool.tile([P, R], F32)
    nc.vector.reduce_sum(out=sq_t, in_=dsq, axis=mybir.AxisListType.X)

    e_t = spool.tile([P, R], F32)
    nc.scalar.activation(e_t, sq_t, mybir.ActivationFunctionType.Exp,
                         scale=EXP_SCALE)

    nc.sync.dma_start(out=e_out, in_=e_t)


_NC_CACHE = {}


def _get_nc():
    if "nc" not in _NC_CACHE:
        _NC_CACHE["nc"] = build_nc()
    return _NC_CACHE["nc"]


def _shard(x, c):
    """Core c's input: [b_local*8 + n, C*HW] (b-major partition order)."""
    xs = x[:, c * B_SH:(c + 1) * B_SH]              # [N, B_SH, C, H, W]
    return np.ascontiguousarray(
        xs.transpose(1, 0, 2, 3, 4).reshape(P, CF))


def kernel(x: np.ndarray):
    """Full-input entry point: x [8, 128, 512, 7, 7] f32 -> (direct, det, logdet)."""
    x = np.asarray(x, dtype=np.float32)
    assert x.shape == (N, B, C, 7, 7), x.shape
    nc = _get_nc()

    in_maps = [{"x": _shard(x, c)} for c in range(N_CORES)]
    res = run_bass_kernel_spmd(nc, in_maps, core_ids=list(range(N_CORES)))

    # e_out rows are (b_local, i); col r-1 holds exp(-g*sq(i, (i+r)%N, b)).
    # Assemble snm[i, (i+r)%N] = mean over all batches; diagonal stays 0.
    rows = np.repeat(np.arange(N), N - 1)
    cols = (np.arange(N)[:, None] + np.arange(1, N)[None, :]).ravel() % N
    acc = np.zeros((N, N), dtype=np.float64)
    for c in range(N_CORES):
        e = res.results[c]["e_out"].astype(np.float64)  # [128, 7]
        s = e.reshape(B_SH, N, N - 1).sum(axis=0)       # [i, r-1]
        acc[rows, cols] += s.ravel()
    snm = acc / B

    direct_div = snm.sum()
    det = np.linalg.det(snm)
    det_div = -det
    sign, logabs = np.linalg.slogdet(snm)
    logdet_div = -logabs if sign > 0 else np.float64(np.nan)

    return (
        np.float32(direct_div),
        np.float32(det_div),
        np.float32(logdet_div),
    )


# revision 23
# speedup vs baseline: 1.0679x; 1.0174x over previous
"""Trainium2 Bass kernel for nn_BranchDiversity.

Computes, for x: [n=8, B=128, C=512, H=7, W=7]:
  xm   = mean(x, axis=C)                       -> [n, B, H, W]
  sq   = sum((xm_i - xm_j)^2, axis=(H,W))      -> [n, n, B]
  snm  = mean(exp(-GAMMA*sq), axis=B) * (1-I)  -> [n, n]
  out  = (sum(snm), -det(snm), -logdet(snm) if det>0 else nan)

Sharding: data-parallel over B across 8 NeuronCores (16 batch each).
Per core the kernel does the memory-bound channel-sum reduction
([128 partitions = (b_local, n) b-major, 25088 free = (C, H*W)]; tapered
chunks so the last exposed reduce is short), then the pairwise tail fully
on-chip: stream_shuffle rotates branches within each batch's 8-partition
group (r=1..7), so sq(i, (i+r)%8, b) comes from per-partition sub/sq/reduce
with no cross-partition DMA. Emits exp(-GAMMA*sq)[(b_local, i), r-1] =
[128, 7]. Host assembles snm (mean over all 128 batches, diag 0) and does
the tiny 8x8 det/logdet.
"""

import numpy as np
from contextlib import ExitStack

import concourse.bass as bass
import concourse.tile as tile
from concourse import bacc, mybir
from concourse.bass_utils import run_bass_kernel_spmd

# Problem constants (hardcoded per contract; kernel.py must be self-contained)
N_CORES = 8
N = 8          # branches
B = 128        # batch
C = 512        # channels
HW = 49        # H*W = 7*7
B_SH = B // N_CORES        # 16 batch per core
P = N * B_SH               # 128 partitions = (b_local, n), b-major
CF = C * HW                # 25088 free elems per partition
GAMMA = 10.0
# exp(-GAMMA * sq_ref) with sq_ref = sq_rawsum / C^2 (xm kept as raw channel
# sums on device; 1/C^2 folded into the activation scale, exact: C^2 = 2^18)
EXP_SCALE = -GAMMA / (C * C)

# Channel chunking: big chunks for DMA efficiency, tapering trailing chunks
# so the final (exposed, serial) reduces on the critical path are short.
# Taper chosen by searching the cost-model chain (see dev notes).
CHUNKS = [48] * 7 + [24] * 3 + [16] + [12] * 4 + [10] + [8] * 3 + [6]
assert sum(CHUNKS) == C

F32 = mybir.dt.float32


def build_nc():
    """Build the per-core Bass program (SPMD: same program, different data)."""
    nc = bacc.Bacc("TRN2", target_bir_lowering=False)
    x = nc.dram_tensor("x", [P, CF], F32, kind="ExternalInput")
    e_out = nc.dram_tensor("e_out", [P, N // 2], F32, kind="ExternalOutput")

    with tile.TileContext(nc) as tc, ExitStack() as ctx:
        _body(ctx, tc, x.ap(), e_out.ap())
    nc.compile()
    return nc


def _body(ctx, tc, x, e_out):
    nc = tc.nc

    xpool = ctx.enter_context(tc.tile_pool(name="xin", bufs=4))
    ppool = ctx.enter_context(tc.tile_pool(name="parts", bufs=2))
    spool = ctx.enter_context(tc.tile_pool(name="small", bufs=1))

    # ---- Stage 1: channel-sum reduction (memory bound) ----
    # Running accumulation: reduce chunk -> part, add into acc (adds hidden
    # under DMA; only the last reduce+add are on the critical path).
    xm_sum = spool.tile([P, HW], F32)
    coff = 0
    for k, cc in enumerate(CHUNKS):
        xt = xpool.tile([P, cc * HW], F32, tag=f"xt{cc}")
        nc.sync.dma_start(out=xt, in_=x[:, coff * HW:(coff + cc) * HW])
        # view [p, hw, c] (c strided by HW) and reduce innermost (c)
        xt_v = xt.rearrange("p (c h) -> p h c", c=cc)
        if k == 0:
            nc.vector.reduce_sum(out=xm_sum, in_=xt_v,
                                 axis=mybir.AxisListType.X)
        else:
            part = ppool.tile([P, HW], F32, tag="part")
            nc.vector.reduce_sum(out=part, in_=xt_v,
                                 axis=mybir.AxisListType.X)
            nc.vector.tensor_add(xm_sum, xm_sum, part)
        coff += cc

    # ---- Stage 2: pairwise tail (tiny, all on-chip) ----
    # Partition layout is (b, n) b-major: each batch's 8 branch rows sit
    # within one 32-partition stream_shuffle quadrant, so rotating branches
    # by r gives xs[(b,i), :] = xm[(b,(i+r)%8), :] without any DMA.
    # sq is symmetric (bitwise: (a-b)^2 == (b-a)^2), so rotations 1..4 cover
    # all 28 unordered pairs (r=4 band twice); host mirrors the rest.
    R = N // 2  # rotations 1..4
    xs = spool.tile([P, R, HW], F32)
    for r in range(1, R + 1):
        mask = [(i & 24) | ((i + r) & 7) for i in range(32)]
        nc.vector.stream_shuffle(xs[:, r - 1, :], xm_sum, mask)

    # A[(b,i), (r,hw)] = xm[(b,i), hw] broadcast over r (stride-0 free dim)
    a_ap = xm_sum[:]
    A = bass.AP(tensor=a_ap.tensor, offset=a_ap.offset,
                ap=[list(a_ap.ap[0]), [0, R], [1, HW]])

    d = spool.tile([P, R, HW], F32)
    nc.vector.tensor_sub(d, A, xs)
    dsq = spool.tile([P, R, HW], F32)
    nc.vector.tensor_mul(dsq, d, d)

    sq_t = spool.tile([P, R], F32)
    nc.vector.reduce_sum(out=sq_t, in_=dsq, axis=mybir.AxisListType.X)

    e_t = spool.tile([P, R], F32)
    nc.scalar.activation(e_t, sq_t, mybir.ActivationFunctionType.Exp,
                         scale=EXP_SCALE)

    nc.sync.dma_start(out=e_out, in_=e_t)


_NC_CACHE = {}


def _get_nc():
    if "nc" not in _NC_CACHE:
        _NC_CACHE["nc"] = build_nc()
    return _NC_CACHE["nc"]


def _shard(x, c):
    """Core c's input: [b_local*8 + n, C*HW] (b-major partition order)."""
    xs = x[:, c * B_SH:(c + 1) * B_SH]              # [N, B_SH, C, H, W]
    return np.ascontiguousarray(
        xs.transpose(1, 0, 2, 3, 4).reshape(P, CF))


def kernel(x: np.ndarray):
    """Full-input entry point: x [8, 128, 512, 7, 7] f32 -> (direct, det, logdet)."""
    x = np.asarray(x, dtype=np.float32)
    assert x.shape == (N, B, C, 7, 7), x.shape
    nc = _get_nc()

    in_maps = [{"x": _shard(x, c)} for c in range(N_CORES)]
    res = run_bass_kernel_spmd(nc, in_maps, core_ids=list(range(N_CORES)))

    # e_out rows are (b_local, i); col r-1 holds exp(-g*sq(i, (i+r)%N, b))
    # for r=1..N/2. Assemble those bands, then mirror (sq is symmetric, so
    # the transposed entries are bitwise identical). Diagonal stays 0.
    R = N // 2
    rows = np.repeat(np.arange(N), R)
    cols = (np.arange(N)[:, None] + np.arange(1, R + 1)[None, :]).ravel() % N
    acc = np.zeros((N, N), dtype=np.float64)
    for c in range(N_CORES):
        e = res.results[c]["e_out"].astype(np.float64)  # [128, R]
        s = e.reshape(B_SH, N, R).sum(axis=0)           # [i, r-1]
        acc[rows, cols] += s.ravel()
    empty = acc == 0.0
    acc[empty] = acc.T[empty]
    snm = acc / B

    direct_div = snm.sum()
    det = np.linalg.det(snm)
    det_div = -det
    sign, logabs = np.linalg.slogdet(snm)
    logdet_div = -logabs if sign > 0 else np.float64(np.nan)

    return (
        np.float32(direct_div),
        np.float32(det_div),
        np.float32(logdet_div),
    )
